# revision 2
# baseline (speedup 1.0000x reference)
"""DeepGMM Trainium2 kernel — mk-parallel over 8 NeuronCores.

Math: out[b,m,k] = w_mk * (-0.5*(quad + D*log2pi) - logdet_mk), where
quad = ||L^-1 (f_b - mu)||^2, f = relu(x@W+b).
Let A' = sqrt(0.5 w) L^-1, c' = A' mu, z = A' f.
quad' = 0.5 w quad = ||z - c'||^2 = S1 + yneg + kappa,
  S1 = sum z^2, yneg = <f, h'> (h' = -2 A'^T c'), kappa = ||c'||^2.
out = gamma - (S1 + yneg), gamma = beta - kappa,
  beta = -0.5 w D log2pi - w logdet.

Each core handles 10 of the 80 (m,k) pairs with full batch B=4096.
Device: feats GEMM (bf16) -> per pair z-GEMM streaming [A'^T | h'] (the
extra column gives yneg for free) -> square+reduce on ACT/DVE.
"""
import sys
import types

sys.path.insert(0, "/opt/trn_rl_repo")


def _install_ntff_shim():
    # The axon boot looks for antenv.axon_hooks to register its NTFF
    # profiling hook; this image's antenv lacks the module, so provide it.
    if "antenv.axon_hooks" in sys.modules:
        return
    mod = types.ModuleType("antenv.axon_hooks")
    holder = [None]
    mod.set_axon_ntff_profile_hook = lambda h: holder.__setitem__(0, h)
    mod.get_axon_ntff_profile_hook = lambda: holder[0]
    sys.modules["antenv.axon_hooks"] = mod
    try:
        import antenv
        antenv.axon_hooks = mod
    except ImportError:
        pass
    # sitecustomize's boot() ran before this shim existed, so its hook
    # registration was skipped; redo it here if the boot module is around.
    try:
        from trn_agent_boot.trn_boot import _ntff_profile_via_ctypes
        hk = _ntff_profile_via_ctypes("/opt/axon/libaxon_pjrt.so")
        if hk is not None:
            mod.set_axon_ntff_profile_hook(hk)
    except Exception:
        pass


_install_ntff_shim()

import numpy as np
import ml_dtypes

B, D_IN, D_F = 4096, 1024, 256
M, K = 10, 8
NPAIR = 80
NCORE = 8
PPC = NPAIR // NCORE  # pairs per core
LOG2PI = float(np.log(2.0 * np.pi))
BF16 = ml_dtypes.bfloat16

_cache = {}


def _build_module():
    import concourse.bass as bass
    import concourse.tile as tile
    import concourse.mybir as mybir
    from concourse import bacc

    dt = mybir.dt
    AF = mybir.ActivationFunctionType
    ALU = mybir.AluOpType

    nc = bacc.Bacc("TRN2", target_bir_lowering=False, debug=False,
                   enable_asserts=False, num_devices=NCORE)

    x_d = nc.dram_tensor("x_in", [128, 8, B], dt.bfloat16, kind="ExternalInput").ap()
    w_d = nc.dram_tensor("w_in", [128, 8, D_F], dt.bfloat16, kind="ExternalInput").ap()
    b_d = nc.dram_tensor("b_in", [128, 2], dt.float32, kind="ExternalInput").ap()
    rhs_d = nc.dram_tensor("rhs_in", [128, PPC, 2, 257], dt.bfloat16,
                           kind="ExternalInput").ap()
    gam_d = nc.dram_tensor("gam_in", [1, PPC], dt.float32, kind="ExternalInput").ap()
    out_d = nc.dram_tensor("out", [B, PPC], dt.float32, kind="ExternalOutput").ap()

    NB = B // 128  # 32 b-blocks
    NC_CH = 8      # x chunks of 512
    GROUPS = [(0, 4), (4, 4), (8, 2)]

    with tile.TileContext(nc) as tc:
        with (
            tc.tile_pool(name="const", bufs=1) as constp,
            tc.tile_pool(name="xin", bufs=3) as xp,
            tc.tile_pool(name="feat", bufs=1) as fp,
            tc.tile_pool(name="junk", bufs=4) as jp,
            tc.tile_pool(name="stat", bufs=4) as sp,
            tc.tile_pool(name="outp", bufs=4) as op,
            tc.tile_pool(name="ps", bufs=2, space="PSUM") as pp,
        ):
            w_sb = constp.tile([128, 8, D_F], dt.bfloat16)
            nc.sync.dma_start(w_sb[:], w_d[:])
            b_sb = constp.tile([128, 2], dt.float32)
            nc.sync.dma_start(b_sb[:], b_d[:])
            rhs_sb = constp.tile([128, PPC, 2, 257], dt.bfloat16)
            nc.sync.dma_start(rhs_sb[:], rhs_d[:])
            gam_sb = constp.tile([1, PPC], dt.float32)
            nc.sync.dma_start(gam_sb[:], gam_d[:])
            ones_sb = constp.tile([1, 128], dt.bfloat16)
            nc.vector.memset(ones_sb[:], 1.0)
            gam_bf = constp.tile([1, PPC], dt.bfloat16)
            nc.vector.tensor_copy(gam_bf[:], gam_sb[:])

            # gamma broadcast [128, PPC] via ones (x) gamma outer product
            gps = pp.tile([128, 512], dt.float32, tag="ps")
            nc.tensor.matmul(gps[:, 0:PPC], lhsT=ones_sb[:], rhs=gam_bf[:],
                             start=True, stop=True)
            gbc = constp.tile([128, PPC], dt.float32)
            nc.vector.tensor_copy(gbc[:], gps[:, 0:PPC])

            # Phase A: featsT (bf16) [2 fblocks][8 chunks of 512]
            fts = [[None] * NC_CH for _ in range(2)]
            for ch in range(NC_CH):
                xc = xp.tile([128, 8, 512], dt.bfloat16, tag="xc")
                nc.sync.dma_start(xc[:], x_d[:, :, ch * 512:(ch + 1) * 512])
                for fb in range(2):
                    ps = pp.tile([128, 512], dt.float32, tag="ps")
                    for kb in range(8):
                        nc.tensor.matmul(
                            ps[:], lhsT=w_sb[:, kb, fb * 128:(fb + 1) * 128],
                            rhs=xc[:, kb, :], start=(kb == 0), stop=(kb == 7))
                    ft = fp.tile([128, 512], dt.bfloat16, tag=f"ft{fb}_{ch}")
                    nc.scalar.activation(ft[:], ps[:], AF.Relu,
                                         bias=b_sb[:, fb:fb + 1])
                    fts[fb][ch] = ft

            # Phase B: per b-block, per pair z-GEMM + square-reduce
            for bb in range(NB):
                ch, off = bb // 4, (bb % 4) * 128
                l1 = fts[0][ch][:, off:off + 128]
                l2 = fts[1][ch][:, off:off + 128]
                pre = sp.tile([128, PPC], dt.float32, tag="pre")
                for (p0, npair) in GROUPS:
                    pz = pp.tile([128, 2048], dt.float32, tag="ps")
                    s1g = sp.tile([128, npair], dt.float32, tag="s1g")
                    for s in range(npair):
                        p = p0 + s
                        sl = pz[:, s * 512:s * 512 + 257]
                        nc.tensor.matmul(sl, lhsT=l1, rhs=rhs_sb[:, p, 0, :],
                                         start=True, stop=True)
                        nc.tensor.matmul(pz[:, s * 512 + 128:s * 512 + 257],
                                         lhsT=l2, rhs=rhs_sb[:, p, 1, 128:257],
                                         start=False, stop=True,
                                         skip_group_check=True)
                    for s in range(npair):
                        zsl = pz[:, s * 512:s * 512 + 256]
                        if True:  # ACT square path (DVE can't 2x-read PSUM)
                            jt = jp.tile([128, 256], dt.bfloat16, tag="ja")
                            nc.scalar.activation(jt[:], zsl, AF.Square,
                                                 accum_out=s1g[:, s:s + 1])
                        else:
                            jt = jp.tile([128, 256], dt.bfloat16, tag="jd")
                            nc.vector.tensor_tensor_reduce(
                                out=jt[:], in0=zsl, in1=zsl, scale=1.0,
                                scalar=0.0, op0=ALU.mult, op1=ALU.add,
                                accum_out=s1g[:, s:s + 1])
                    aug = pz[:, 0:npair * 512].rearrange(
                        "p (s x) -> p s x", x=512)[:, :, 256]
                    nc.vector.tensor_tensor(pre[:, p0:p0 + npair], aug,
                                            s1g[:], op=ALU.add)
                ot = op.tile([128, PPC], dt.float32, tag="ot")
                nc.vector.tensor_sub(ot[:], gbc[:], pre[:])
                nc.sync.dma_start(out_d[bb * 128:(bb + 1) * 128, :], ot[:])
    nc.finalize()
    return nc


def _prep_inputs(x, W, b, means, covs, weights):
    # host: shard/cast/layout + small per-pair parameter preprocessing
    x = np.asarray(x, np.float32)
    W = np.asarray(W, np.float32)
    b = np.asarray(b, np.float32)
    means = np.asarray(means, np.float32).reshape(NPAIR, D_F)
    covs = np.asarray(covs, np.float32).reshape(NPAIR, D_F, D_F)
    weights = np.asarray(weights, np.float32)

    x_in = np.ascontiguousarray(
        x.T.reshape(8, 128, B).transpose(1, 0, 2)).astype(BF16)
    w_in = np.ascontiguousarray(
        W.reshape(8, 128, D_F).transpose(1, 0, 2)).astype(BF16)
    b_in = np.ascontiguousarray(b.reshape(2, 128).T).astype(np.float32)

    ew = np.exp(weights - weights.max(axis=1, keepdims=True))
    w_sm = (ew / ew.sum(axis=1, keepdims=True)).reshape(NPAIR)

    from scipy.linalg import solve_triangular
    rhs_all = np.zeros((NPAIR, D_F, 257), np.float32)
    gam_all = np.zeros(NPAIR, np.float32)
    eye = np.eye(D_F, dtype=np.float32)
    for q in range(NPAIR):
        L = np.tril(covs[q])
        A = solve_triangular(L, eye, lower=True)
        s = np.sqrt(0.5 * w_sm[q])
        Ap = s * A
        cp = Ap @ means[q]
        hp = -2.0 * (Ap.T @ cp)
        logdet = np.log(np.diag(L)).sum()
        beta = -0.5 * w_sm[q] * D_F * LOG2PI - w_sm[q] * logdet
        gam_all[q] = beta - float(cp @ cp)
        rhs_all[q, :, 0:256] = Ap.T
        rhs_all[q, :, 256] = hp

    in_maps = []
    for c in range(NCORE):
        sl = slice(c * PPC, (c + 1) * PPC)
        rhs_c = np.ascontiguousarray(
            rhs_all[sl].reshape(PPC, 2, 128, 257).transpose(2, 0, 1, 3)
        ).astype(BF16)
        gam_c = gam_all[sl].reshape(1, PPC).astype(np.float32)
        in_maps.append({
            "x_in": x_in, "w_in": w_in, "b_in": b_in,
            "rhs_in": rhs_c, "gam_in": gam_c,
        })
    return in_maps


def kernel(x, W, b, means, covs, weights, _want_trace=False):
    from concourse import bass_utils

    if "nc" not in _cache:
        _cache["nc"] = _build_module()
    nc = _cache["nc"]
    in_maps = _prep_inputs(x, W, b, means, covs, weights)
    res = bass_utils.run_bass_kernel_spmd(
        nc, in_maps, core_ids=list(range(NCORE)), trace=_want_trace)
    if _want_trace:
        _cache["last_results"] = res
    out = np.concatenate([res.results[c]["out"] for c in range(NCORE)],
                         axis=1)
    return np.ascontiguousarray(out.reshape(B, M, K).astype(np.float32))



# revision 12
# speedup vs baseline: 1.1687x; 1.1687x over previous
"""DeepGMM Trainium2 kernel — batch-parallel over 8 NeuronCores.

Math: out[b,m,k] = w_mk * (-0.5*(quad + D*log2pi) - logdet_mk), where
quad = ||L^-1 (f_b - mu)||^2, f = relu(x@W+b).
Let A' = sqrt(0.5 w) L^-1 (lower tri), Q = A'^T A', c' = A' mu,
h' = -2 A'^T c'.  Then
  out = gamma - (quad' + yneg),
  quad' = f^T Q f,  yneg = <f, h'>,
  gamma = -||c'||^2 - w*(0.5*D*log2pi + logdet).

Each core takes B/8 = 512 batch rows and all 80 (m,k) pairs.
quad' is computed two ways, split across engines to balance them:
  - z-pairs (ACT): z^T = F^T A'^T via PE (A'^T upper-tri: 1.5 matmuls),
    then ACT Square with accum_out -> sum z^2.
  - G-pairs (DVE): G'^T = F^T U via PE (U = triu(Q)+triu(Q,1), upper
    tri), then DVE tensor_tensor_reduce(F^T * G'^T) with a single PSUM
    operand (in0 = F^T from SBUF) -> sum F.G' = f^T Q f.
Both schemes share the PE structure (rhs top [128,256] + bot [128,128])
and keep F stationary (lhsT), so LDWEIGHTS is 2 per group of 4 pairs.
yneg for all pairs comes from one small GEMM against -h'; gamma is DMA'd
replicated and folded in the same DVE op.
"""
import sys
import types

sys.path.insert(0, "/opt/trn_rl_repo")


def _install_ntff_shim():
    # The axon boot looks for antenv.axon_hooks to register its NTFF
    # profiling hook; this image's antenv lacks the module, so provide it.
    if "antenv.axon_hooks" in sys.modules:
        return
    mod = types.ModuleType("antenv.axon_hooks")
    holder = [None]
    mod.set_axon_ntff_profile_hook = lambda h: holder.__setitem__(0, h)
    mod.get_axon_ntff_profile_hook = lambda: holder[0]
    sys.modules["antenv.axon_hooks"] = mod
    try:
        import antenv
        antenv.axon_hooks = mod
    except ImportError:
        pass
    # sitecustomize's boot() ran before this shim existed, so its hook
    # registration was skipped; redo it here if the boot module is around.
    try:
        from trn_agent_boot.trn_boot import _ntff_profile_via_ctypes
        hk = _ntff_profile_via_ctypes("/opt/axon/libaxon_pjrt.so")
        if hk is not None:
            mod.set_axon_ntff_profile_hook(hk)
    except Exception:
        pass


_install_ntff_shim()

import numpy as np
import ml_dtypes

B, D_IN, D_F = 4096, 1024, 256
M, K = 10, 8
NPAIR = 80
NCORE = 8
BPC = B // NCORE          # batch rows per core = 512
NBB = BPC // 128          # b-blocks per core = 4
NDUO = NPAIR // 2         # 40
NZ = 80                   # pairs 0..NZ-1 via ACT square; rest via DVE
LOG2PI = float(np.log(2.0 * np.pi))
BF16 = ml_dtypes.bfloat16

_cache = {}


def _build_module():
    import concourse.bass as bass
    import concourse.tile as tile
    import concourse.mybir as mybir
    from concourse import bacc

    dt = mybir.dt
    AF = mybir.ActivationFunctionType
    ALU = mybir.AluOpType

    nc = bacc.Bacc("TRN2", target_bir_lowering=False, debug=False,
                   enable_asserts=False, num_devices=NCORE)

    x_d = nc.dram_tensor("x_in", [128, 8, BPC], dt.bfloat16,
                         kind="ExternalInput").ap()
    w_d = nc.dram_tensor("w_in", [128, 8, D_F], dt.bfloat16,
                         kind="ExternalInput").ap()
    b_d = nc.dram_tensor("b_in", [128, 2], dt.float32,
                         kind="ExternalInput").ap()
    rhs_d = nc.dram_tensor("rhs_in", [128, NDUO, 768], dt.bfloat16,
                           kind="ExternalInput").ap()
    hneg_d = nc.dram_tensor("hneg_in", [128, 2, NPAIR], dt.bfloat16,
                            kind="ExternalInput").ap()
    gam_d = nc.dram_tensor("gam_in", [128, NPAIR], dt.float32,
                           kind="ExternalInput").ap()
    brow_d = nc.dram_tensor("brow_in", [1, D_F], dt.bfloat16,
                            kind="ExternalInput").ap()
    out_d = nc.dram_tensor("out", [BPC, NPAIR], dt.float32,
                           kind="ExternalOutput").ap()

    NZD = NZ // 2              # z duos
    RHS_CHUNK = 5              # duos per rhs DMA

    # groups of up to 2 duos, z and G kept separate (separate psum rings)
    def make_groups(d0, d1):
        return [list(range(d, min(d + 2, d1))) for d in range(d0, d1, 2)]

    zgroups = make_groups(0, NZD)
    ggroups = make_groups(NZD, NDUO)
    # interleave so ACT and DVE are both fed early
    groups = []
    zi = gi = 0
    while zi < len(zgroups) or gi < len(ggroups):
        take_g = (gi * (len(zgroups) + 1) <= zi * (len(ggroups) + 1))
        if gi < len(ggroups) and (take_g or zi >= len(zgroups)):
            groups.append(("g", ggroups[gi])); gi += 1
        else:
            groups.append(("z", zgroups[zi])); zi += 1

    with tile.TileContext(nc) as tc:
        with (
            tc.tile_pool(name="const", bufs=1) as constp,
            tc.tile_pool(name="junk", bufs=4) as jp,
            tc.tile_pool(name="stat", bufs=2) as sp,
            tc.tile_pool(name="outp", bufs=2) as op,
            tc.tile_pool(name="psz", bufs=2, space="PSUM") as psz,
            tc.tile_pool(name="psg", bufs=2, space="PSUM") as psg,
        ):
            w_sb = constp.tile([128, 8, D_F], dt.bfloat16)
            nc.sync.dma_start(w_sb[:], w_d[:])
            b_sb = constp.tile([128, 2], dt.float32)
            nc.sync.dma_start(b_sb[:], b_d[:])
            brow_sb = constp.tile([1, D_F], dt.bfloat16)
            nc.sync.dma_start(brow_sb[:], brow_d[:])
            ones_sb = constp.tile([1, 128], dt.bfloat16)
            nc.vector.memset(ones_sb[:], 1.0)
            hneg_sb = constp.tile([128, 2, NPAIR], dt.bfloat16)
            nc.sync.dma_start(hneg_sb[:], hneg_d[:])
            gam_sb = constp.tile([128, NPAIR], dt.float32)
            nc.sync.dma_start(gam_sb[:], gam_d[:])
            xc = constp.tile([128, 8, BPC], dt.bfloat16)
            nc.sync.dma_start(xc[:], x_d[:])
            rhs_sb = constp.tile([128, NDUO, 768], dt.bfloat16)
            for d0 in range(0, NDUO, RHS_CHUNK):
                d1 = min(d0 + RHS_CHUNK, NDUO)
                nc.sync.dma_start(rhs_sb[:, d0:d1, :], rhs_d[:, d0:d1, :])

            # ---- Phase A: feats^T = relu(W^T x^T + b) ----
            ft = []
            for fb in range(2):
                ps = psz.tile([128, 1024], dt.float32, tag="zg")
                for kb in range(8):
                    nc.tensor.matmul(
                        ps[:, 0:BPC],
                        lhsT=w_sb[:, kb, fb * 128:(fb + 1) * 128],
                        rhs=xc[:, kb, :], start=(kb == 0), stop=(kb == 7))
                f = constp.tile([128, BPC], dt.bfloat16, tag=f"ft{fb}")
                nc.scalar.activation(f[:], ps[:, 0:BPC], AF.Relu,
                                     bias=b_sb[:, fb:fb + 1])
                ft.append(f)

            # feats in [batch, fdim] layout for the DVE in0 operand,
            # via a second GEMM with x stationary (bias added by ones-row)
            ftT = constp.tile([128, NBB, D_F], dt.bfloat16)
            for bb in range(NBB):
                pt = psg.tile([128, 1024], dt.float32, tag="gg")
                for kb in range(8):
                    nc.tensor.matmul(
                        pt[:, 0:D_F],
                        lhsT=xc[:, kb, bb * 128:(bb + 1) * 128],
                        rhs=w_sb[:, kb, :], start=(kb == 0), stop=(kb == 7))
                nc.tensor.matmul(pt[:, 0:D_F], lhsT=ones_sb[:],
                                 rhs=brow_sb[:], start=False, stop=True,
                                 skip_group_check=True)
                nc.scalar.activation(ftT[:, bb, :], pt[:, 0:D_F], AF.Relu)

            # gyn[b, q] = gamma[q] - yneg[b, q], per b-block
            gyn = constp.tile([128, NBB, NPAIR], dt.float32)
            for bb in range(NBB):
                py = psz.tile([128, 1024], dt.float32, tag="zg")
                l1 = ft[0][:, bb * 128:(bb + 1) * 128]
                l2 = ft[1][:, bb * 128:(bb + 1) * 128]
                nc.tensor.matmul(py[:, 0:NPAIR], lhsT=l1,
                                 rhs=hneg_sb[:, 0, :], start=True, stop=True)
                nc.tensor.matmul(py[:, 0:NPAIR], lhsT=l2,
                                 rhs=hneg_sb[:, 1, :], start=False, stop=True,
                                 skip_group_check=True)
                nc.vector.tensor_tensor(gyn[:, bb, :], gam_sb[:],
                                        py[:, 0:NPAIR], op=ALU.add)

            # ---- Phase B: per b-block, per pair quad' ----
            for bb in range(NBB):
                l1 = ft[0][:, bb * 128:(bb + 1) * 128]
                l2 = ft[1][:, bb * 128:(bb + 1) * 128]
                f_bb = ftT[:, bb, :]
                s1 = sp.tile([128, NPAIR], dt.float32, tag="s1")
                for kind, duos in groups:
                    pool, tag = (psz, "zg") if kind == "z" else (psg, "gg")
                    pz = pool.tile([128, 1024], dt.float32, tag=tag)
                    for j, d in enumerate(duos):
                        nc.tensor.matmul(
                            pz[:, j * 512:j * 512 + 512], lhsT=l1,
                            rhs=rhs_sb[:, d, 0:512], start=True, stop=True)
                    for j, d in enumerate(duos):
                        nc.tensor.matmul(
                            pz[:, j * 512 + 128:j * 512 + 256], lhsT=l2,
                            rhs=rhs_sb[:, d, 512:640], start=False, stop=True,
                            skip_group_check=True)
                        nc.tensor.matmul(
                            pz[:, j * 512 + 384:j * 512 + 512], lhsT=l2,
                            rhs=rhs_sb[:, d, 640:768], start=False, stop=True,
                            skip_group_check=True)
                    for j, d in enumerate(duos):
                        for h in range(2):
                            p = 2 * d + h
                            zsl = pz[:, j * 512 + h * 256:
                                     j * 512 + h * 256 + 256]
                            if kind == "z":
                                jt = jp.tile([128, 256], dt.bfloat16,
                                             tag="ja")
                                nc.scalar.activation(
                                    jt[:], zsl, AF.Square,
                                    accum_out=s1[:, p:p + 1])
                            else:
                                jt = jp.tile([128, 256], dt.bfloat16,
                                             tag="jd")
                                nc.vector.tensor_tensor_reduce(
                                    out=jt[:], in0=f_bb, in1=zsl,
                                    scale=1.0, scalar=0.0,
                                    op0=ALU.mult, op1=ALU.add,
                                    accum_out=s1[:, p:p + 1])
                ot = op.tile([128, NPAIR], dt.float32, tag="ot")
                nc.vector.tensor_tensor(ot[:], gyn[:, bb, :], s1[:],
                                        op=ALU.subtract)
                nc.sync.dma_start(out_d[bb * 128:(bb + 1) * 128, :], ot[:])
    nc.finalize()
    return nc


def _prep_inputs(x, W, b, means, covs, weights):
    # host: shard/cast/layout + small per-pair parameter preprocessing
    x = np.asarray(x, np.float32)
    W = np.asarray(W, np.float32)
    b = np.asarray(b, np.float32)
    means = np.asarray(means, np.float32).reshape(NPAIR, D_F)
    covs = np.asarray(covs, np.float32).reshape(NPAIR, D_F, D_F)
    weights = np.asarray(weights, np.float32)

    xt = np.ascontiguousarray(
        x.T.reshape(8, 128, B).transpose(1, 0, 2)).astype(BF16)
    w_in = np.ascontiguousarray(
        W.reshape(8, 128, D_F).transpose(1, 0, 2)).astype(BF16)
    b_in = np.ascontiguousarray(b.reshape(2, 128).T).astype(np.float32)
    brow_in = b.reshape(1, D_F).astype(BF16)

    ew = np.exp(weights - weights.max(axis=1, keepdims=True))
    w_sm = (ew / ew.sum(axis=1, keepdims=True)).reshape(NPAIR)

    from scipy.linalg import solve_triangular
    rhs_all = np.zeros((NDUO, 128, 768), np.float32)
    hneg_all = np.zeros((128, 2, NPAIR), np.float32)
    gam_all = np.zeros(NPAIR, np.float32)
    eye = np.eye(D_F, dtype=np.float32)
    for q in range(NPAIR):
        L = np.tril(covs[q])
        A = solve_triangular(L, eye, lower=True)
        s = np.sqrt(0.5 * w_sm[q])
        Ap = s * A
        cp = Ap @ means[q]
        hp = -2.0 * (Ap.T @ cp)
        logdet = np.log(np.diag(L)).sum()
        gam_all[q] = (-float(cp @ cp)
                      - w_sm[q] * (0.5 * D_F * LOG2PI + logdet))
        hneg_all[:, 0, q] = -hp[0:128]
        hneg_all[:, 1, q] = -hp[128:256]
        if q < NZ:
            R = Ap.T
        else:
            Q = Ap.T @ Ap
            R = np.triu(Q) + np.triu(Q, 1)
        d, h = q // 2, q % 2
        rhs_all[d, :, h * 256:h * 256 + 256] = R[0:128, :]
        rhs_all[d, :, 512 + h * 128:512 + h * 128 + 128] = R[128:256, 128:256]

    rhs_in = np.ascontiguousarray(
        rhs_all.transpose(1, 0, 2)).astype(BF16)
    hneg_in = hneg_all.astype(BF16)
    gam_in = np.ascontiguousarray(
        np.broadcast_to(gam_all[None, :], (128, NPAIR))).astype(np.float32)

    in_maps = []
    for c in range(NCORE):
        x_in = np.ascontiguousarray(xt[:, :, c * BPC:(c + 1) * BPC])
        in_maps.append({
            "x_in": x_in, "w_in": w_in, "b_in": b_in, "rhs_in": rhs_in,
            "hneg_in": hneg_in, "gam_in": gam_in, "brow_in": brow_in,
        })
    return in_maps


def kernel(x, W, b, means, covs, weights, _want_trace=False):
    from concourse import bass_utils

    if "nc" not in _cache:
        _cache["nc"] = _build_module()
    nc = _cache["nc"]
    in_maps = _prep_inputs(x, W, b, means, covs, weights)
    res = bass_utils.run_bass_kernel_spmd(
        nc, in_maps, core_ids=list(range(NCORE)), trace=_want_trace)
    if _want_trace:
        _cache["last_results"] = res
    out = np.concatenate([res.results[c]["out"] for c in range(NCORE)],
                         axis=0)
    return np.ascontiguousarray(out.reshape(B, M, K).astype(np.float32))


# revision 16
# speedup vs baseline: 2.2729x; 1.9449x over previous
"""DeepGMM Trainium2 kernel — batch-parallel over 8 NeuronCores.

Math: out[b,m,k] = w_mk * (-0.5*(quad + D*log2pi) - logdet_mk), where
quad = ||L^-1 (f_b - mu)||^2, f = relu(x@W+b).
Let A' = sqrt(0.5 w) L^-1 (lower tri), Q = A'^T A', c' = A' mu,
h' = -2 A'^T c'.  Then
  out = gamma - (quad' + yneg),
  quad' = f^T Q f,  yneg = <f, h'>,
  gamma = -||c'||^2 - w*(0.5*D*log2pi + logdet).

Each core takes B/8 = 512 batch rows and all 80 (m,k) pairs.
quad' is computed two ways, split across engines to balance them:
  - z-pairs (ACT): z^T = F^T A'^T via PE (A'^T upper-tri: 1.5 matmuls),
    then ACT Square with accum_out -> sum z^2.
  - G-pairs (DVE): G'^T = F^T U via PE (U = triu(Q)+triu(Q,1), upper
    tri), then DVE tensor_tensor_reduce(F^T * G'^T) with a single PSUM
    operand (in0 = F^T from SBUF) -> sum F.G' = f^T Q f.
Both schemes share the PE structure (rhs top [128,256] + bot [128,128])
and keep F stationary (lhsT), so LDWEIGHTS is 2 per group of 4 pairs.
yneg for all pairs comes from one small GEMM against -h'; gamma is DMA'd
replicated and folded in the same DVE op.
"""
import sys
import types

sys.path.insert(0, "/opt/trn_rl_repo")


def _install_ntff_shim():
    # The axon boot looks for antenv.axon_hooks to register its NTFF
    # profiling hook; this image's antenv lacks the module, so provide it.
    if "antenv.axon_hooks" in sys.modules:
        return
    mod = types.ModuleType("antenv.axon_hooks")
    holder = [None]
    mod.set_axon_ntff_profile_hook = lambda h: holder.__setitem__(0, h)
    mod.get_axon_ntff_profile_hook = lambda: holder[0]
    sys.modules["antenv.axon_hooks"] = mod
    try:
        import antenv
        antenv.axon_hooks = mod
    except ImportError:
        pass
    # sitecustomize's boot() ran before this shim existed, so its hook
    # registration was skipped; redo it here if the boot module is around.
    try:
        from trn_agent_boot.trn_boot import _ntff_profile_via_ctypes
        hk = _ntff_profile_via_ctypes("/opt/axon/libaxon_pjrt.so")
        if hk is not None:
            mod.set_axon_ntff_profile_hook(hk)
    except Exception:
        pass


_install_ntff_shim()

import numpy as np
import ml_dtypes

B, D_IN, D_F = 4096, 1024, 256
M, K = 10, 8
NPAIR = 80
NCORE = 8
BPC = B // NCORE          # batch rows per core = 512
NBB = BPC // 128          # b-blocks per core = 4
NDUO = NPAIR // 2         # 40
NZ = 30                   # pairs 0..NZ-1 via ACT square; rest via DVE
LOG2PI = float(np.log(2.0 * np.pi))
BF16 = ml_dtypes.bfloat16

_cache = {}


def _build_module():
    import concourse.bass as bass
    import concourse.tile as tile
    import concourse.mybir as mybir
    from concourse import bacc

    dt = mybir.dt
    AF = mybir.ActivationFunctionType
    ALU = mybir.AluOpType

    nc = bacc.Bacc("TRN2", target_bir_lowering=False, debug=False,
                   enable_asserts=False, num_devices=NCORE)

    x_d = nc.dram_tensor("x_in", [128, 8, BPC], dt.bfloat16,
                         kind="ExternalInput").ap()
    w_d = nc.dram_tensor("w_in", [128, 8, D_F], dt.bfloat16,
                         kind="ExternalInput").ap()
    b_d = nc.dram_tensor("b_in", [128, 2], dt.float32,
                         kind="ExternalInput").ap()
    rhs_d = nc.dram_tensor("rhs_in", [128, NDUO, 768], dt.bfloat16,
                           kind="ExternalInput").ap()
    hneg_d = nc.dram_tensor("hneg_in", [128, 2, NPAIR], dt.bfloat16,
                            kind="ExternalInput").ap()
    gam_d = nc.dram_tensor("gam_in", [128, NPAIR], dt.float32,
                           kind="ExternalInput").ap()
    brow_d = nc.dram_tensor("brow_in", [1, D_F], dt.bfloat16,
                            kind="ExternalInput").ap()
    out_d = nc.dram_tensor("out", [BPC, NPAIR], dt.float32,
                           kind="ExternalOutput").ap()

    NZD = NZ // 2              # z duos
    RHS_CHUNK = 5              # duos per rhs DMA

    # groups of up to 2 duos, z and G kept separate (separate psum rings)
    def make_groups(d0, d1):
        return [list(range(d, min(d + 2, d1))) for d in range(d0, d1, 2)]

    zgroups = make_groups(0, NZD)
    ggroups = make_groups(NZD, NDUO)
    # interleave so ACT and DVE are both fed early
    groups = []
    zi = gi = 0
    while zi < len(zgroups) or gi < len(ggroups):
        take_g = (gi * (len(zgroups) + 1) <= zi * (len(ggroups) + 1))
        if gi < len(ggroups) and (take_g or zi >= len(zgroups)):
            groups.append(("g", ggroups[gi])); gi += 1
        else:
            groups.append(("z", zgroups[zi])); zi += 1

    with tile.TileContext(nc) as tc:
        with (
            tc.tile_pool(name="const", bufs=1) as constp,
            tc.tile_pool(name="junk", bufs=4) as jp,
            tc.tile_pool(name="stat", bufs=2) as sp,
            tc.tile_pool(name="outp", bufs=2) as op,
            tc.tile_pool(name="psz", bufs=2, space="PSUM") as psz,
            tc.tile_pool(name="psg", bufs=2, space="PSUM") as psg,
        ):
            w_sb = constp.tile([128, 8, D_F], dt.bfloat16)
            nc.sync.dma_start(w_sb[:], w_d[:])
            b_sb = constp.tile([128, 2], dt.float32)
            nc.sync.dma_start(b_sb[:], b_d[:])
            brow_sb = constp.tile([1, D_F], dt.bfloat16)
            nc.sync.dma_start(brow_sb[:], brow_d[:])
            ones_sb = constp.tile([1, 128], dt.bfloat16)
            nc.vector.memset(ones_sb[:], 1.0)
            hneg_sb = constp.tile([128, 2, NPAIR], dt.bfloat16)
            nc.sync.dma_start(hneg_sb[:], hneg_d[:])
            gam_sb = constp.tile([128, NPAIR], dt.float32)
            nc.sync.dma_start(gam_sb[:], gam_d[:])
            xc = constp.tile([128, 8, BPC], dt.bfloat16)
            nc.sync.dma_start(xc[:], x_d[:])
            rhs_sb = constp.tile([128, NDUO, 768], dt.bfloat16)
            for d0 in range(0, NDUO, RHS_CHUNK):
                d1 = min(d0 + RHS_CHUNK, NDUO)
                nc.sync.dma_start(rhs_sb[:, d0:d1, :], rhs_d[:, d0:d1, :])

            # ---- Phase A: feats^T = relu(W^T x^T + b) ----
            ft = []
            for fb in range(2):
                ps = psz.tile([128, 1024], dt.float32, tag="zg")
                for kb in range(8):
                    nc.tensor.matmul(
                        ps[:, 0:BPC],
                        lhsT=w_sb[:, kb, fb * 128:(fb + 1) * 128],
                        rhs=xc[:, kb, :], start=(kb == 0), stop=(kb == 7))
                f = constp.tile([128, BPC], dt.bfloat16, tag=f"ft{fb}")
                nc.scalar.activation(f[:], ps[:, 0:BPC], AF.Relu,
                                     bias=b_sb[:, fb:fb + 1])
                ft.append(f)

            # feats in [batch, fdim] layout for the DVE in0 operand,
            # via a second GEMM with x stationary (bias added by ones-row)
            ftT = constp.tile([128, NBB, D_F], dt.float32)
            for bb in range(NBB):
                pt = psg.tile([128, 1024], dt.float32, tag="gg")
                for kb in range(8):
                    nc.tensor.matmul(
                        pt[:, 0:D_F],
                        lhsT=xc[:, kb, bb * 128:(bb + 1) * 128],
                        rhs=w_sb[:, kb, :], start=(kb == 0), stop=(kb == 7))
                nc.tensor.matmul(pt[:, 0:D_F], lhsT=ones_sb[:],
                                 rhs=brow_sb[:], start=False, stop=True,
                                 skip_group_check=True)
                nc.scalar.activation(ftT[:, bb, :], pt[:, 0:D_F], AF.Relu)

            # gyn[b, q] = gamma[q] - yneg[b, q], per b-block
            gyn = constp.tile([128, NBB, NPAIR], dt.float32)
            for bb in range(NBB):
                py = psz.tile([128, 1024], dt.float32, tag="zg")
                l1 = ft[0][:, bb * 128:(bb + 1) * 128]
                l2 = ft[1][:, bb * 128:(bb + 1) * 128]
                nc.tensor.matmul(py[:, 0:NPAIR], lhsT=l1,
                                 rhs=hneg_sb[:, 0, :], start=True, stop=True)
                nc.tensor.matmul(py[:, 0:NPAIR], lhsT=l2,
                                 rhs=hneg_sb[:, 1, :], start=False, stop=True,
                                 skip_group_check=True)
                nc.vector.tensor_tensor(gyn[:, bb, :], gam_sb[:],
                                        py[:, 0:NPAIR], op=ALU.add)

            # ---- Phase B: per b-block, per pair quad' ----
            for bb in range(NBB):
                l1 = ft[0][:, bb * 128:(bb + 1) * 128]
                l2 = ft[1][:, bb * 128:(bb + 1) * 128]
                f_bb = ftT[:, bb, :]
                s1 = sp.tile([128, NPAIR], dt.float32, tag="s1")
                for kind, duos in groups:
                    pool, tag = (psz, "zg") if kind == "z" else (psg, "gg")
                    pz = pool.tile([128, 1024], dt.float32, tag=tag)
                    for j, d in enumerate(duos):
                        nc.tensor.matmul(
                            pz[:, j * 512:j * 512 + 512], lhsT=l1,
                            rhs=rhs_sb[:, d, 0:512], start=True, stop=True)
                    for j, d in enumerate(duos):
                        nc.tensor.matmul(
                            pz[:, j * 512 + 128:j * 512 + 256], lhsT=l2,
                            rhs=rhs_sb[:, d, 512:640], start=False, stop=True,
                            skip_group_check=True)
                        nc.tensor.matmul(
                            pz[:, j * 512 + 384:j * 512 + 512], lhsT=l2,
                            rhs=rhs_sb[:, d, 640:768], start=False, stop=True,
                            skip_group_check=True)
                    for j, d in enumerate(duos):
                        for h in range(2):
                            p = 2 * d + h
                            zsl = pz[:, j * 512 + h * 256:
                                     j * 512 + h * 256 + 256]
                            if kind == "z":
                                jt = jp.tile([128, 256], dt.bfloat16,
                                             tag="ja")
                                nc.scalar.activation(
                                    jt[:], zsl, AF.Square,
                                    accum_out=s1[:, p:p + 1])
                            else:
                                jt = jp.tile([128, 256], dt.float32,
                                             tag="jd")
                                nc.vector.scalar_tensor_tensor(
                                    out=jt[:], in0=f_bb, scalar=1.0,
                                    in1=zsl, op0=ALU.mult, op1=ALU.mult,
                                    accum_out=s1[:, p:p + 1])
                ot = op.tile([128, NPAIR], dt.float32, tag="ot")
                nc.vector.tensor_tensor(ot[:], gyn[:, bb, :], s1[:],
                                        op=ALU.subtract)
                nc.sync.dma_start(out_d[bb * 128:(bb + 1) * 128, :], ot[:])
    nc.finalize()
    return nc


def _prep_inputs(x, W, b, means, covs, weights):
    # host: shard/cast/layout + small per-pair parameter preprocessing
    x = np.asarray(x, np.float32)
    W = np.asarray(W, np.float32)
    b = np.asarray(b, np.float32)
    means = np.asarray(means, np.float32).reshape(NPAIR, D_F)
    covs = np.asarray(covs, np.float32).reshape(NPAIR, D_F, D_F)
    weights = np.asarray(weights, np.float32)

    xt = np.ascontiguousarray(
        x.T.reshape(8, 128, B).transpose(1, 0, 2)).astype(BF16)
    w_in = np.ascontiguousarray(
        W.reshape(8, 128, D_F).transpose(1, 0, 2)).astype(BF16)
    b_in = np.ascontiguousarray(b.reshape(2, 128).T).astype(np.float32)
    brow_in = b.reshape(1, D_F).astype(BF16)

    ew = np.exp(weights - weights.max(axis=1, keepdims=True))
    w_sm = (ew / ew.sum(axis=1, keepdims=True)).reshape(NPAIR)

    from scipy.linalg import solve_triangular
    rhs_all = np.zeros((NDUO, 128, 768), np.float32)
    hneg_all = np.zeros((128, 2, NPAIR), np.float32)
    gam_all = np.zeros(NPAIR, np.float32)
    eye = np.eye(D_F, dtype=np.float32)
    for q in range(NPAIR):
        L = np.tril(covs[q])
        A = solve_triangular(L, eye, lower=True)
        s = np.sqrt(0.5 * w_sm[q])
        Ap = s * A
        cp = Ap @ means[q]
        hp = -2.0 * (Ap.T @ cp)
        logdet = np.log(np.diag(L)).sum()
        gam_all[q] = (-float(cp @ cp)
                      - w_sm[q] * (0.5 * D_F * LOG2PI + logdet))
        hneg_all[:, 0, q] = -hp[0:128]
        hneg_all[:, 1, q] = -hp[128:256]
        if q < NZ:
            R = Ap.T
        else:
            Q = Ap.T @ Ap
            R = np.triu(Q) + np.triu(Q, 1)
        d, h = q // 2, q % 2
        rhs_all[d, :, h * 256:h * 256 + 256] = R[0:128, :]
        rhs_all[d, :, 512 + h * 128:512 + h * 128 + 128] = R[128:256, 128:256]

    rhs_in = np.ascontiguousarray(
        rhs_all.transpose(1, 0, 2)).astype(BF16)
    hneg_in = hneg_all.astype(BF16)
    gam_in = np.ascontiguousarray(
        np.broadcast_to(gam_all[None, :], (128, NPAIR))).astype(np.float32)

    in_maps = []
    for c in range(NCORE):
        x_in = np.ascontiguousarray(xt[:, :, c * BPC:(c + 1) * BPC])
        in_maps.append({
            "x_in": x_in, "w_in": w_in, "b_in": b_in, "rhs_in": rhs_in,
            "hneg_in": hneg_in, "gam_in": gam_in, "brow_in": brow_in,
        })
    return in_maps


def kernel(x, W, b, means, covs, weights, _want_trace=False):
    from concourse import bass_utils

    if "nc" not in _cache:
        _cache["nc"] = _build_module()
    nc = _cache["nc"]
    in_maps = _prep_inputs(x, W, b, means, covs, weights)
    res = bass_utils.run_bass_kernel_spmd(
        nc, in_maps, core_ids=list(range(NCORE)), trace=_want_trace)
    if _want_trace:
        _cache["last_results"] = res
    out = np.concatenate([res.results[c]["out"] for c in range(NCORE)],
                         axis=0)
    return np.ascontiguousarray(out.reshape(B, M, K).astype(np.float32))
